# revision 7
# baseline (speedup 1.0000x reference)
"""Trainium2 Bass kernel for nn_CTRPredictor (gnn_message_passing).

score[e] = dot(normalize(x[src[e]]), normalize(x[dst[e]]))  for E edges.

Strategy v3 (8 NeuronCores, SPMD, no collectives, no normalize phase):
  - Nodes in 4 quarters Q0..Q3 (25000 each).  Each core's DRAM holds a
    raw bf16 half-table of two quarters as two 25088-row windows (A, B)
    (int16 window-local gather indices).  Host routes edges to cores by
    quarter pair: 6 off-diag cores own the cross pairs (edges oriented
    src->A, dst->B), 2 diag cores own within-quarter edges of {Q0,Q1} /
    {Q2,Q3} as (A,A)+(B,B).
  - Universal chunk schedule [AB,AA,BB,AB]x5 (20 chunks x 8192 edges);
    per chunk two HBM dma_gather calls (src rows, dst rows) on 4 SWDGE
    queues.  Per-call valid counts are data-driven via Pool reg_load +
    trailing -1 idxs, so a core's unused combo chunks cost only a
    128-row dummy gather.  Gathers start immediately - no table prep.
  - Per chunk: ACT squares + DVE grouped reduces give per-slot |u|^2,
    |v|^2; DVE mul + grouped reduce gives u.v; score = u.v *
    rsqrt(|u|^2*|v|^2) (ACT sqrt + DVE reciprocal + DVE mul).
  - Host un-permutes scores back to edge order.
"""

import numpy as np

N = 100000
D = 128
E = 640000
CORES = 8
NQ = 4
Q = N // NQ               # 25000 nodes per quarter
WTOK = 25088              # row slots per window (196*128)
CHUNK = 8192              # edges per chunk
NCHUNK = 20               # chunks (universal schedule)
NCALL = 2 * NCHUNK        # 40 gather calls
ICOL = CHUNK // 16        # 512 idx columns per call
BLK = CHUNK // 128        # 64 score columns per chunk
SCOL = NCHUNK * BLK       # 1280 score columns
PADV = 128                # min valid idxs per call (dummy fill)

# universal combo schedule: (src_win, dst_win) per chunk.  Order chosen so
# each combo's calls land on all 4 SWDGE queues (queue = call_index % 4).
COMBOS = [(0, 1), (0, 0), (1, 1), (0, 1)] * 5

# per-core quarters (window A, window B); cores 6,7 take the diagonals
PAIRS = [(0, 1), (0, 2), (0, 3), (1, 2), (1, 3), (2, 3), (0, 1), (2, 3)]
OFFDIAG_CORE = {frozenset(p): i for i, p in enumerate(PAIRS[:6])}
DIAG_CORE = {0: 6, 1: 6, 2: 7, 3: 7}

_CACHE = {}
LAST_RESULTS = None
RUN_KWARGS = {}  # extra kwargs for run_bass_kernel_spmd (used by test harness)


def _build():
    from concourse import bass, bacc, tile, mybir

    f32 = mybir.dt.float32
    bf16 = mybir.dt.bfloat16
    i16 = mybir.dt.int16
    i32 = mybir.dt.int32

    nc = bacc.Bacc("TRN2", target_bir_lowering=False, debug=False,
                   num_devices=CORES, num_swdge_queues=4,
                   dynamic_dma_scratch_size=40960)

    xt_d = nc.dram_tensor("xt", [2 * WTOK, D], bf16, kind="ExternalInput")
    idx_d = nc.dram_tensor("eidx", [128, NCALL * ICOL], i16,
                           kind="ExternalInput")
    cnt_d = nc.dram_tensor("cnt", [1, NCALL], i32, kind="ExternalInput")
    out_d = nc.dram_tensor("out", [128, SCOL], f32, kind="ExternalOutput")

    wins = [xt_d.ap()[:WTOK, :], xt_d.ap()[WTOK:, :]]

    with tile.TileContext(nc) as tc:
        with tc.tile_pool(name="pp", bufs=1) as pp, \
             tc.tile_pool(name="gp", bufs=3) as gp, \
             tc.tile_pool(name="sp", bufs=2) as sp, \
             tc.tile_pool(name="ip", bufs=2) as ip, \
             tc.tile_pool(name="np_", bufs=4) as npool:

            cnt = pp.tile([1, NCALL], i32)
            nc.sync.dma_start(out=cnt[:, :], in_=cnt_d.ap())
            score = pp.tile([128, SCOL], f32)

            creg = nc.gpsimd.alloc_register("cnt_reg")
            NIP = 5                    # chunks per idx stream piece
            IPW = 2 * NIP * ICOL       # idx cols per piece
            ixt = None
            for ch in range(NCHUNK):
                if ch % NIP == 0:
                    ixt = ip.tile([128, IPW], i16, tag="ix")
                    p0 = ch * 2 * ICOL
                    nc.sync.dma_start(out=ixt[:, :],
                                      in_=idx_d.ap()[:, p0:p0 + IPW])
                sw, dw = COMBOS[ch]
                g = gp.tile([128, 2 * CHUNK], bf16, tag="g")
                for half, win in ((0, sw), (1, dw)):
                    call = 2 * ch + half
                    ic0 = (call - (ch // NIP) * 2 * NIP) * ICOL
                    nc.gpsimd.reg_load(creg, cnt[0:1, call:call + 1])
                    nc.gpsimd.dma_gather(
                        out_ap=g[:, half * CHUNK:(half + 1) * CHUNK]
                        .rearrange("p (c d) -> p c d", d=D),
                        in_ap=wins[win],
                        idxs_ap=ixt[:, ic0:ic0 + ICOL],
                        num_idxs=CHUNK, num_idxs_reg=creg, elem_size=D,
                        single_packet=False, queue_num=call % 4,
                    )
                # per-slot squared norms on ACT + DVE grouped reduce
                nn = npool.tile([128, 2 * BLK], f32, tag="nn")
                for half in range(2):
                    sq = sp.tile([128, CHUNK], bf16, tag="sq")
                    nc.scalar.activation(
                        out=sq[:, :],
                        in_=g[:, half * CHUNK:(half + 1) * CHUNK],
                        func=mybir.ActivationFunctionType.Square)
                    nc.vector.tensor_reduce(
                        out=nn[:, half * BLK:(half + 1) * BLK],
                        in_=sq[:, :].rearrange("p (c d) -> p c d", d=D),
                        axis=mybir.AxisListType.X,
                        op=mybir.AluOpType.add,
                    )
                # u.v
                nc.vector.tensor_mul(out=g[:, :CHUNK], in0=g[:, :CHUNK],
                                     in1=g[:, CHUNK:])
                raw = npool.tile([128, BLK], f32, tag="raw")
                nc.vector.tensor_reduce(
                    out=raw[:, :],
                    in_=g[:, :CHUNK].rearrange("p (c d) -> p c d", d=D),
                    axis=mybir.AxisListType.X,
                    op=mybir.AluOpType.add,
                )
                # score = raw * rsqrt(|u|^2 * |v|^2)
                nprod = npool.tile([128, BLK], f32, tag="npr")
                nc.vector.tensor_mul(out=nprod[:, :], in0=nn[:, :BLK],
                                     in1=nn[:, BLK:])
                nc.scalar.activation(out=nprod[:, :], in_=nprod[:, :],
                                     func=mybir.ActivationFunctionType.Sqrt)
                rin = npool.tile([128, BLK], f32, tag="rin")
                nc.vector.reciprocal(out=rin[:, :], in_=nprod[:, :])
                nc.vector.tensor_mul(
                    out=score[:, ch * BLK:(ch + 1) * BLK],
                    in0=raw[:, :], in1=rin[:, :])

            nc.sync.dma_start(out=out_d.ap(), in_=score[:, :])

    nc.compile()
    return nc


def _prepare_core(core, x_bf, src_l, dst_l, sq, dq):
    """Build one core's half-table, idx tiles, counts, and inverse map.

    src_l/dst_l: the core's edges' endpoints (already oriented so that
    src quarter == PAIRS[core][0] for off-diag cores).  sq/dq: quarter of
    each (oriented) endpoint.
    """
    qa, qb = PAIRS[core]
    ne = src_l.size

    # window w holds quarter (qa, qb)[w]; window-local row = node - q*Q
    xt = np.zeros((2 * WTOK, D), dtype=x_bf.dtype)
    for w, q in enumerate((qa, qb)):
        xt[w * WTOK:w * WTOK + Q] = x_bf[q * Q:(q + 1) * Q]
        xt[w * WTOK + Q:(w + 1) * WTOK, 0] = 1.0  # pad rows (never gathered)

    sloc = (src_l - sq * Q).astype(np.int64)
    dloc = (dst_l - dq * Q).astype(np.int64)

    wmap = np.full(NQ, -1, dtype=np.int64)
    wmap[qa], wmap[qb] = 0, 1
    swin, dwin = wmap[sq], wmap[dq]
    assert np.all(swin >= 0) and np.all(dwin >= 0)
    ckey = swin * 2 + dwin            # 0=AA 1=AB 3=BB (2=BA must not occur)
    assert not np.any(ckey == 2), "unoriented BA edge"

    chunks_of = {1: [i for i, c in enumerate(COMBOS) if c == (0, 1)],
                 0: [i for i, c in enumerate(COMBOS) if c == (0, 0)],
                 3: [i for i, c in enumerate(COMBOS) if c == (1, 1)]}

    callidx = np.full((NCALL, CHUNK), -1, dtype=np.int16)
    counts = np.zeros(NCALL, dtype=np.int32)
    rows = np.empty(ne, dtype=np.int64)
    cols = np.empty(ne, dtype=np.int64)

    for key, chl in chunks_of.items():
        ids = np.nonzero(ckey == key)[0]
        cap = len(chl) * CHUNK
        if ids.size > cap:
            raise ValueError(f"combo {key} overflow: {ids.size} > {cap}")
        nch = len(chl)
        base, rem = divmod(ids.size, nch)
        off = 0
        for k, ch in enumerate(chl):
            take = base + (1 if k < rem else 0)
            sel = ids[off:off + take]
            off += take
            # sort by src row for gather locality
            sel = sel[np.argsort(sloc[sel], kind="stable")]
            v = sel.size
            nv = max(v, PADV)
            scall, dcall = 2 * ch, 2 * ch + 1
            callidx[scall, :v] = sloc[sel]
            callidx[dcall, :v] = dloc[sel]
            callidx[scall, v:nv] = 0
            callidx[dcall, v:nv] = 0
            counts[scall] = counts[dcall] = nv
            j = np.arange(v)
            rows[sel] = j % 128
            cols[sel] = ch * BLK + j // 128

    # wrap idx: element i of a call -> [i%16, call*ICOL + i//16], x8 replicate
    wrapped = callidx.reshape(NCALL, ICOL, 16).transpose(0, 2, 1)
    eidx16 = wrapped.transpose(1, 0, 2).reshape(16, NCALL * ICOL)
    eidx = np.tile(eidx16, (8, 1))

    return xt, np.ascontiguousarray(eidx), counts, rows, cols


def kernel(x, src, dst):
    global LAST_RESULTS
    import ml_dtypes
    from concourse.bass_utils import run_bass_kernel_spmd

    if "nc" not in _CACHE:
        _CACHE["nc"] = _build()
    nc = _CACHE["nc"]

    x_bf = np.asarray(x, dtype=np.float32).astype(ml_dtypes.bfloat16)
    src_i = np.asarray(src).astype(np.int64)
    dst_i = np.asarray(dst).astype(np.int64)

    qs = src_i // Q
    qd = dst_i // Q

    core_of = np.empty(E, dtype=np.int64)
    swap = np.zeros(E, dtype=bool)
    offdiag = qs != qd
    dmask = ~offdiag
    diag_lut = np.array([DIAG_CORE[q] for q in range(NQ)], dtype=np.int64)
    core_of[dmask] = diag_lut[qs[dmask]]
    for key, c in OFFDIAG_CORE.items():
        a, b = sorted(key)
        m = offdiag & (np.minimum(qs, qd) == a) & (np.maximum(qs, qd) == b)
        core_of[m] = c
        swap[m] = qs[m] != a        # orient src into window A's quarter
    ssrc = np.where(swap, dst_i, src_i)
    sdst = np.where(swap, src_i, dst_i)
    sqs = np.where(swap, qd, qs)
    sqd = np.where(swap, qs, qd)

    in_maps = []
    inv = []
    for i in range(CORES):
        m = np.nonzero(core_of == i)[0]
        xt, eidx, counts, rows, cols = _prepare_core(
            i, x_bf, ssrc[m], sdst[m], sqs[m], sqd[m])
        inv.append((m, rows, cols))
        in_maps.append({
            "xt": xt,
            "eidx": eidx,
            "cnt": np.ascontiguousarray(counts.reshape(1, NCALL)),
        })

    res = run_bass_kernel_spmd(nc, in_maps, core_ids=list(range(CORES)),
                               **RUN_KWARGS)
    LAST_RESULTS = res

    out = np.empty(E, dtype=np.float32)
    for i in range(CORES):
        tilev = np.asarray(res.results[i]["out"])
        m, rows, cols = inv[i]
        out[m] = tilev[rows, cols]
    return out.reshape(E, 1)


# revision 10
# speedup vs baseline: 1.1767x; 1.1767x over previous
"""Trainium2 Bass kernel for nn_CTRPredictor (gnn_message_passing).

score[e] = dot(normalize(x[src[e]]), normalize(x[dst[e]]))  for E edges.

Strategy v4 (8 NeuronCores, SPMD, no collectives):
  - Nodes in 4 quarters Q0..Q3 (25000 each).  Each core gets a raw bf16
    half-table (two quarters) packed [128, 392*128] (window w on
    partitions w*64..w*64+63; token local id l at partition w*64+l//392,
    256B column l%392).  The core normalizes rows in SBUF (ACT square,
    DVE reduce, sqrt, reciprocal, DVE scale) and writes the result with
    one contiguous DMA to a DRAM table whose flat [50176, 128] row view
    makes window w = rows [w*25088, (w+1)*25088) (gather idx = p*392+c,
    int16-safe per window).
  - Edge routing: 6 off-diag cores own cross quarter pairs (edges
    oriented src->A, dst->B); 2 diag cores own within-quarter edges of
    {Q0,Q1} / {Q2,Q3} with windows = quarter *halves* interleaved, so
    their profile is AB:<=5 + AA:<=3 + BB:<=3 chunks.
  - Dense universal schedule of 16 chunks x 8192 edges (AB x10 + AA x3 +
    BB x3); per chunk two HBM dma_gather calls (src, dst) into separate
    tiles on 4 SWDGE queues; per-call valid counts via Pool reg_load +
    trailing -1 idxs make a core's unused combo chunks nearly free.
    Chunk edges sorted by (src 4096-row band, dst row) for HBM row
    locality on both gather streams.
  - Per chunk: DVE mul + grouped reduce -> scores (table prenormalized).
  - Host un-permutes scores back to edge order.
"""

import numpy as np

N = 100000
D = 128
E = 640000
CORES = 8
NQ = 4
Q = N // NQ               # 25000 nodes per quarter
WTOK = 25088              # row slots per window (64 partitions * 392 cols)
TCOL = 392                # 256B columns per partition in the packed table
HTOK = 12544              # half-quarter slot base inside a diag window
CHUNK = 8192              # edges per chunk
NCHUNK = 16               # chunks (universal schedule)
NCALL = 2 * NCHUNK        # 32 gather calls
ICOL = CHUNK // 16        # 512 idx columns per call
BLK = CHUNK // 128        # 64 score columns per chunk
SCOL = NCHUNK * BLK       # 1024 score columns
PADV = 128                # min valid idxs per call (dummy fill)
SBAND = 4096              # src band size for chunk locality sort

AB, AA, BB = (0, 1), (0, 0), (1, 1)
# 16-chunk universal schedule; parity-balanced so each combo's calls hit
# all 4 SWDGE queues (chunk even -> queues 0,1; odd -> queues 2,3).
COMBOS = [AB, AB, AA, AB, AB, AA, AB, BB, AB, BB, AA, AB, BB, AB, AB, AB]

# per-core quarters (window A, window B); cores 6,7 take the diagonals
PAIRS = [(0, 1), (0, 2), (0, 3), (1, 2), (1, 3), (2, 3), (0, 1), (2, 3)]
OFFDIAG_CORE = {frozenset(p): i for i, p in enumerate(PAIRS[:6])}
DIAG_CORE = {0: 6, 1: 6, 2: 7, 3: 7}

_CACHE = {}
LAST_RESULTS = None
RUN_KWARGS = {}  # extra kwargs for run_bass_kernel_spmd (used by test harness)


def _build():
    from concourse import bass, bacc, tile, mybir

    f32 = mybir.dt.float32
    bf16 = mybir.dt.bfloat16
    i16 = mybir.dt.int16
    i32 = mybir.dt.int32

    nc = bacc.Bacc("TRN2", target_bir_lowering=False, debug=False,
                   num_devices=CORES, num_swdge_queues=4,
                   dynamic_dma_scratch_size=40960)

    xt_d = nc.dram_tensor("xt", [128, TCOL * D], bf16, kind="ExternalInput")
    idx_d = nc.dram_tensor("eidx", [128, NCALL * ICOL], i16,
                           kind="ExternalInput")
    cnt_d = nc.dram_tensor("cnt", [1, NCALL], i32, kind="ExternalInput")
    out_d = nc.dram_tensor("out", [128, SCOL], f32, kind="ExternalOutput")

    NLCH = 8                      # normalize column chunks
    LC = TCOL // NLCH             # 49 cols per chunk

    with tile.TileContext(nc) as tc:
        with tc.tile_pool(name="dram", bufs=1, space="DRAM") as dp, \
             tc.tile_pool(name="pp", bufs=1) as pp:

            cnt = pp.tile([1, NCALL], i32)
            nc.sync.dma_start(out=cnt[:, :], in_=cnt_d.ap())
            score = pp.tile([128, SCOL], f32)

            ntab = dp.tile([2 * WTOK, D], bf16, name="ntab")
            ntab_pk = ntab[:, :].rearrange("(p c) d -> p (c d)", p=128)

            # ---- phase 0: load packed half-table, normalize, write back ----
            with tc.tile_pool(name="ph0", bufs=1) as p0, \
                 tc.tile_pool(name="sqp", bufs=2) as sqp:
                tab = p0.tile([128, TCOL * D], bf16)
                ns = p0.tile([128, TCOL], f32)
                rns = p0.tile([128, TCOL], f32)
                for lc in range(NLCH):
                    c0 = lc * LC
                    seg = tab[:, c0 * D:(c0 + LC) * D]
                    nc.sync.dma_start(
                        out=seg, in_=xt_d.ap()[:, c0 * D:(c0 + LC) * D])
                    sq = sqp.tile([128, LC * D], bf16, tag="sq")
                    nc.scalar.activation(
                        out=sq[:, :], in_=seg,
                        func=mybir.ActivationFunctionType.Square)
                    nc.vector.tensor_reduce(
                        out=ns[:, c0:c0 + LC],
                        in_=sq[:, :].rearrange("p (r d) -> p r d", d=D),
                        axis=mybir.AxisListType.X,
                        op=mybir.AluOpType.add,
                    )
                nc.scalar.activation(out=ns[:, :], in_=ns[:, :],
                                     func=mybir.ActivationFunctionType.Sqrt)
                nc.vector.reciprocal(out=rns[:, :], in_=ns[:, :])
                for lc in range(NLCH):
                    c0 = lc * LC
                    nc.vector.tensor_mul(
                        out=tab[:, c0 * D:(c0 + LC) * D].rearrange(
                            "p (r d) -> p r d", d=D),
                        in0=tab[:, c0 * D:(c0 + LC) * D].rearrange(
                            "p (r d) -> p r d", d=D),
                        in1=rns[:, c0:c0 + LC].unsqueeze(-1).to_broadcast(
                            [128, LC, D]),
                    )
                    nc.sync.dma_start(
                        out=ntab_pk[:, c0 * D:(c0 + LC) * D],
                        in_=tab[:, c0 * D:(c0 + LC) * D])

            wins = [ntab[:WTOK, :], ntab[WTOK:, :]]

            # ---- main loop ----
            creg = nc.gpsimd.alloc_register("cnt_reg")
            NIP = 4                    # chunks per idx stream piece
            IPW = 2 * NIP * ICOL
            ixt = None
            with tc.tile_pool(name="gu", bufs=3) as gu, \
                 tc.tile_pool(name="gv", bufs=3) as gv, \
                 tc.tile_pool(name="ip", bufs=2) as ip, \
                 tc.tile_pool(name="rp", bufs=4) as rp:
                for ch in range(NCHUNK):
                    if ch % NIP == 0:
                        ixt = ip.tile([128, IPW], i16, tag="ix")
                        p0c = ch * 2 * ICOL
                        nc.sync.dma_start(out=ixt[:, :],
                                          in_=idx_d.ap()[:, p0c:p0c + IPW])
                    sw, dw = COMBOS[ch]
                    ut = gu.tile([128, CHUNK], bf16, tag="u")
                    vt = gv.tile([128, CHUNK], bf16, tag="v")
                    for half, win, t in ((0, sw, ut), (1, dw, vt)):
                        call = 2 * ch + half
                        ic0 = (call - (ch // NIP) * 2 * NIP) * ICOL
                        nc.gpsimd.reg_load(creg, cnt[0:1, call:call + 1])
                        nc.gpsimd.dma_gather(
                            out_ap=t[:, :].rearrange("p (c d) -> p c d", d=D),
                            in_ap=wins[win],
                            idxs_ap=ixt[:, ic0:ic0 + ICOL],
                            num_idxs=CHUNK, num_idxs_reg=creg, elem_size=D,
                            single_packet=False, queue_num=call % 4,
                        )
                    nc.vector.tensor_mul(out=ut[:, :], in0=ut[:, :],
                                         in1=vt[:, :])
                    nc.vector.tensor_reduce(
                        out=score[:, ch * BLK:(ch + 1) * BLK],
                        in_=ut[:, :].rearrange("p (c d) -> p c d", d=D),
                        axis=mybir.AxisListType.X,
                        op=mybir.AluOpType.add,
                    )

                nc.sync.dma_start(out=out_d.ap(), in_=score[:, :])

    nc.compile()
    return nc


def _win_local(core, nodes, q):
    """node -> (window, window-local row) for this core's table."""
    qa, qb = PAIRS[core]
    loc = nodes - q * Q
    if core < 6:
        win = np.where(q == qa, 0, 1)
        wloc = loc
    else:
        # diag: window = half parity; local = half base + offset
        half = (loc >= Q // 2).astype(np.int64)
        win = half
        base = np.where(q == qa, 0, HTOK)
        wloc = base + loc - half * (Q // 2)
    return win, wloc


def _prepare_core(core, x_bf, src_l, dst_l, sq, dq):
    """Build one core's packed table, idx tiles, counts, and inverse map."""
    qa, qb = PAIRS[core]
    ne = src_l.size

    # window content: slot l of window w -> node
    wnode = np.full((2, WTOK), -1, dtype=np.int64)
    if core < 6:
        for w, q in enumerate((qa, qb)):
            wnode[w, :Q] = np.arange(q * Q, (q + 1) * Q)
    else:
        h = Q // 2
        for hw in range(2):  # window = half parity
            wnode[hw, :h] = np.arange(qa * Q + hw * h, qa * Q + (hw + 1) * h)
            wnode[hw, HTOK:HTOK + h] = np.arange(
                qb * Q + hw * h, qb * Q + (hw + 1) * h)

    # packed table [128, TCOL*D]: window w slot l -> partition w*64+l//392,
    # col l%392  (so DRAM flat row p*392+c == w*WTOK + l)
    xt = np.zeros((128, TCOL * D), dtype=x_bf.dtype)
    for w in range(2):
        valid = wnode[w] >= 0
        rowdat = np.zeros((WTOK, D), dtype=x_bf.dtype)
        rowdat[valid] = x_bf[wnode[w][valid]]
        rowdat[~valid, 0] = 1.0  # pad rows (never gathered)
        xt[w * 64:(w + 1) * 64, :] = rowdat.reshape(64, TCOL * D)

    swin, sloc = _win_local(core, src_l, sq)
    dwin, dloc = _win_local(core, dst_l, dq)

    ckey = swin * 2 + dwin            # 0=AA 1=AB 3=BB (2=BA: swap to AB)
    ba = ckey == 2
    if np.any(ba):  # orientation within diag windows: swap src/dst
        sloc[ba], dloc[ba] = dloc[ba].copy(), sloc[ba].copy()
        ckey[ba] = 1

    chunks_of = {1: [i for i, c in enumerate(COMBOS) if c == AB],
                 0: [i for i, c in enumerate(COMBOS) if c == AA],
                 3: [i for i, c in enumerate(COMBOS) if c == BB]}

    callidx = np.full((NCALL, CHUNK), -1, dtype=np.int16)
    counts = np.zeros(NCALL, dtype=np.int32)
    rows = np.empty(ne, dtype=np.int64)
    cols = np.empty(ne, dtype=np.int64)

    for key, chl in chunks_of.items():
        ids = np.nonzero(ckey == key)[0]
        cap = len(chl) * CHUNK
        if ids.size > cap:
            raise ValueError(f"combo {key} overflow: {ids.size} > {cap}")
        # (src band, dst row) sort for HBM row locality, then split evenly
        order = np.lexsort((dloc[ids], sloc[ids] // SBAND))
        ids = ids[order]
        nch = len(chl)
        base, rem = divmod(ids.size, nch)
        off = 0
        for k, ch in enumerate(chl):
            take = base + (1 if k < rem else 0)
            sel = ids[off:off + take]
            off += take
            v = sel.size
            nv = max(v, PADV)
            scall, dcall = 2 * ch, 2 * ch + 1
            callidx[scall, :v] = sloc[sel]
            callidx[dcall, :v] = dloc[sel]
            callidx[scall, v:nv] = 0
            callidx[dcall, v:nv] = 0
            counts[scall] = counts[dcall] = nv
            j = np.arange(v)
            rows[sel] = j % 128
            cols[sel] = ch * BLK + j // 128

    # wrap idx: element i of a call -> [i%16, call*ICOL + i//16], x8 replicate
    wrapped = callidx.reshape(NCALL, ICOL, 16).transpose(0, 2, 1)
    eidx16 = wrapped.transpose(1, 0, 2).reshape(16, NCALL * ICOL)
    eidx = np.tile(eidx16, (8, 1))

    return xt, np.ascontiguousarray(eidx), counts, rows, cols


def kernel(x, src, dst):
    global LAST_RESULTS
    import ml_dtypes
    from concourse.bass_utils import run_bass_kernel_spmd

    if "nc" not in _CACHE:
        _CACHE["nc"] = _build()
    nc = _CACHE["nc"]

    x_bf = np.asarray(x, dtype=np.float32).astype(ml_dtypes.bfloat16)
    src_i = np.asarray(src).astype(np.int64)
    dst_i = np.asarray(dst).astype(np.int64)

    qs = src_i // Q
    qd = dst_i // Q

    core_of = np.empty(E, dtype=np.int64)
    swap = np.zeros(E, dtype=bool)
    offdiag = qs != qd
    dmask = ~offdiag
    diag_lut = np.array([DIAG_CORE[q] for q in range(NQ)], dtype=np.int64)
    core_of[dmask] = diag_lut[qs[dmask]]
    for key, c in OFFDIAG_CORE.items():
        a, b = sorted(key)
        m = offdiag & (np.minimum(qs, qd) == a) & (np.maximum(qs, qd) == b)
        core_of[m] = c
        swap[m] = qs[m] != a        # orient src into window A's quarter
    ssrc = np.where(swap, dst_i, src_i)
    sdst = np.where(swap, src_i, dst_i)
    sqs = np.where(swap, qd, qs)
    sqd = np.where(swap, qs, qd)

    in_maps = []
    inv = []
    for i in range(CORES):
        m = np.nonzero(core_of == i)[0]
        xt, eidx, counts, rows, cols = _prepare_core(
            i, x_bf, ssrc[m], sdst[m], sqs[m], sqd[m])
        inv.append((m, rows, cols))
        in_maps.append({
            "xt": xt,
            "eidx": eidx,
            "cnt": np.ascontiguousarray(counts.reshape(1, NCALL)),
        })

    res = run_bass_kernel_spmd(nc, in_maps, core_ids=list(range(CORES)),
                               **RUN_KWARGS)
    LAST_RESULTS = res

    out = np.empty(E, dtype=np.float32)
    for i in range(CORES):
        tilev = np.asarray(res.results[i]["out"])
        m, rows, cols = inv[i]
        out[m] = tilev[rows, cols]
    return out.reshape(E, 1)


# revision 11
# speedup vs baseline: 1.2430x; 1.0564x over previous
"""Trainium2 Bass kernel for nn_CTRPredictor (gnn_message_passing).

score[e] = dot(normalize(x[src[e]]), normalize(x[dst[e]]))  for E edges.

Strategy v4 (8 NeuronCores, SPMD, no collectives):
  - Nodes in 4 quarters Q0..Q3 (25000 each).  Each core gets a raw bf16
    half-table (two quarters) packed [128, 392*128] (window w on
    partitions w*64..w*64+63; token local id l at partition w*64+l//392,
    256B column l%392).  The core normalizes rows in SBUF (ACT square,
    DVE reduce, sqrt, reciprocal, DVE scale) and writes the result with
    one contiguous DMA to a DRAM table whose flat [50176, 128] row view
    makes window w = rows [w*25088, (w+1)*25088) (gather idx = p*392+c,
    int16-safe per window).
  - Edge routing: 6 off-diag cores own cross quarter pairs (edges
    oriented src->A, dst->B); 2 diag cores own within-quarter edges of
    {Q0,Q1} / {Q2,Q3} with windows = quarter *halves* interleaved, so
    their profile is AB:<=5 + AA:<=3 + BB:<=3 chunks.
  - Dense universal schedule of 16 chunks x 8192 edges (AB x10 + AA x3 +
    BB x3); per chunk two HBM dma_gather calls (src, dst) into separate
    tiles on 4 SWDGE queues; per-call valid counts via Pool reg_load +
    trailing -1 idxs make a core's unused combo chunks nearly free.
    Chunk edges sorted by (src 4096-row band, dst row) for HBM row
    locality on both gather streams.
  - Per chunk: DVE mul + grouped reduce -> scores (table prenormalized).
  - Host un-permutes scores back to edge order.
"""

import numpy as np

N = 100000
D = 128
E = 640000
CORES = 8
NQ = 4
Q = N // NQ               # 25000 nodes per quarter
WTOK = 25088              # row slots per window (64 partitions * 392 cols)
TCOL = 392                # 256B columns per partition in the packed table
HTOK = 12544              # half-quarter slot base inside a diag window
CHUNK = 4096              # edges per chunk
NCHUNK = 32               # chunks (universal schedule)
NCALL = 2 * NCHUNK        # 32 gather calls
ICOL = CHUNK // 16        # 512 idx columns per call
BLK = CHUNK // 128        # 64 score columns per chunk
SCOL = NCHUNK * BLK       # 1024 score columns
PADV = 128                # min valid idxs per call (dummy fill)
SBAND = 4096              # src band size for chunk locality sort

AB, AA, BB = (0, 1), (0, 0), (1, 1)
# 32-chunk universal schedule; parity-balanced so each combo's calls hit
# all 4 SWDGE queues (chunk even -> queues 0,1; odd -> queues 2,3).
_AA_AT = {2, 7, 10, 15, 18, 23}
_BB_AT = {4, 9, 12, 17, 20, 25}
COMBOS = [AA if i in _AA_AT else BB if i in _BB_AT else AB for i in range(32)]

# per-core quarters (window A, window B); cores 6,7 take the diagonals
PAIRS = [(0, 1), (0, 2), (0, 3), (1, 2), (1, 3), (2, 3), (0, 1), (2, 3)]
OFFDIAG_CORE = {frozenset(p): i for i, p in enumerate(PAIRS[:6])}
DIAG_CORE = {0: 6, 1: 6, 2: 7, 3: 7}

_CACHE = {}
LAST_RESULTS = None
RUN_KWARGS = {}  # extra kwargs for run_bass_kernel_spmd (used by test harness)


def _build():
    from concourse import bass, bacc, tile, mybir

    f32 = mybir.dt.float32
    bf16 = mybir.dt.bfloat16
    i16 = mybir.dt.int16
    i32 = mybir.dt.int32

    nc = bacc.Bacc("TRN2", target_bir_lowering=False, debug=False,
                   num_devices=CORES, num_swdge_queues=4,
                   dynamic_dma_scratch_size=49152)

    xt_d = nc.dram_tensor("xt", [128, TCOL * D], bf16, kind="ExternalInput")
    idx_d = nc.dram_tensor("eidx", [128, NCALL * ICOL], i16,
                           kind="ExternalInput")
    cnt_d = nc.dram_tensor("cnt", [1, NCALL], i32, kind="ExternalInput")
    out_d = nc.dram_tensor("out", [128, SCOL], f32, kind="ExternalOutput")

    NLCH = 8                      # normalize column chunks
    LC = TCOL // NLCH             # 49 cols per chunk

    with tile.TileContext(nc) as tc:
        with tc.tile_pool(name="dram", bufs=1, space="DRAM") as dp, \
             tc.tile_pool(name="pp", bufs=1) as pp:

            cnt = pp.tile([1, NCALL], i32)
            nc.sync.dma_start(out=cnt[:, :], in_=cnt_d.ap())
            score = pp.tile([128, SCOL], f32)

            ntab = dp.tile([2 * WTOK, D], bf16, name="ntab")
            ntab_pk = ntab[:, :].rearrange("(p c) d -> p (c d)", p=128)

            # ---- phase 0: load packed half-table, normalize, write back ----
            with tc.tile_pool(name="ph0", bufs=1) as p0, \
                 tc.tile_pool(name="sqp", bufs=2) as sqp:
                tab = p0.tile([128, TCOL * D], bf16)
                ns = p0.tile([128, TCOL], f32)
                rns = p0.tile([128, TCOL], f32)
                for lc in range(NLCH):
                    c0 = lc * LC
                    seg = tab[:, c0 * D:(c0 + LC) * D]
                    nc.sync.dma_start(
                        out=seg, in_=xt_d.ap()[:, c0 * D:(c0 + LC) * D])
                    sq = sqp.tile([128, LC * D], bf16, tag="sq")
                    nc.scalar.activation(
                        out=sq[:, :], in_=seg,
                        func=mybir.ActivationFunctionType.Square)
                    nc.vector.tensor_reduce(
                        out=ns[:, c0:c0 + LC],
                        in_=sq[:, :].rearrange("p (r d) -> p r d", d=D),
                        axis=mybir.AxisListType.X,
                        op=mybir.AluOpType.add,
                    )
                nc.scalar.activation(out=ns[:, :], in_=ns[:, :],
                                     func=mybir.ActivationFunctionType.Sqrt)
                nc.vector.reciprocal(out=rns[:, :], in_=ns[:, :])
                for lc in range(NLCH):
                    c0 = lc * LC
                    nc.vector.tensor_mul(
                        out=tab[:, c0 * D:(c0 + LC) * D].rearrange(
                            "p (r d) -> p r d", d=D),
                        in0=tab[:, c0 * D:(c0 + LC) * D].rearrange(
                            "p (r d) -> p r d", d=D),
                        in1=rns[:, c0:c0 + LC].unsqueeze(-1).to_broadcast(
                            [128, LC, D]),
                    )
                    nc.sync.dma_start(
                        out=ntab_pk[:, c0 * D:(c0 + LC) * D],
                        in_=tab[:, c0 * D:(c0 + LC) * D])

            wins = [ntab[:WTOK, :], ntab[WTOK:, :]]

            # ---- main loop ----
            creg = nc.gpsimd.alloc_register("cnt_reg")
            NIP = 8                    # chunks per idx stream piece
            IPW = 2 * NIP * ICOL
            ixt = None
            with tc.tile_pool(name="gu", bufs=5) as gu, \
                 tc.tile_pool(name="gv", bufs=5) as gv, \
                 tc.tile_pool(name="ip", bufs=2) as ip, \
                 tc.tile_pool(name="rp", bufs=4) as rp:
                for ch in range(NCHUNK):
                    if ch % NIP == 0:
                        ixt = ip.tile([128, IPW], i16, tag="ix")
                        p0c = ch * 2 * ICOL
                        nc.sync.dma_start(out=ixt[:, :],
                                          in_=idx_d.ap()[:, p0c:p0c + IPW])
                    sw, dw = COMBOS[ch]
                    ut = gu.tile([128, CHUNK], bf16, tag="u")
                    vt = gv.tile([128, CHUNK], bf16, tag="v")
                    for half, win, t in ((0, sw, ut), (1, dw, vt)):
                        call = 2 * ch + half
                        ic0 = (call - (ch // NIP) * 2 * NIP) * ICOL
                        nc.gpsimd.reg_load(creg, cnt[0:1, call:call + 1])
                        nc.gpsimd.dma_gather(
                            out_ap=t[:, :].rearrange("p (c d) -> p c d", d=D),
                            in_ap=wins[win],
                            idxs_ap=ixt[:, ic0:ic0 + ICOL],
                            num_idxs=CHUNK, num_idxs_reg=creg, elem_size=D,
                            single_packet=False, queue_num=call % 4,
                        )
                    nc.vector.tensor_mul(out=ut[:, :], in0=ut[:, :],
                                         in1=vt[:, :])
                    nc.vector.tensor_reduce(
                        out=score[:, ch * BLK:(ch + 1) * BLK],
                        in_=ut[:, :].rearrange("p (c d) -> p c d", d=D),
                        axis=mybir.AxisListType.X,
                        op=mybir.AluOpType.add,
                    )

                nc.sync.dma_start(out=out_d.ap(), in_=score[:, :])

    nc.compile()
    return nc


def _win_local(core, nodes, q):
    """node -> (window, window-local row) for this core's table."""
    qa, qb = PAIRS[core]
    loc = nodes - q * Q
    if core < 6:
        win = np.where(q == qa, 0, 1)
        wloc = loc
    else:
        # diag: window = half parity; local = half base + offset
        half = (loc >= Q // 2).astype(np.int64)
        win = half
        base = np.where(q == qa, 0, HTOK)
        wloc = base + loc - half * (Q // 2)
    return win, wloc


def _prepare_core(core, x_bf, src_l, dst_l, sq, dq):
    """Build one core's packed table, idx tiles, counts, and inverse map."""
    qa, qb = PAIRS[core]
    ne = src_l.size

    # window content: slot l of window w -> node
    wnode = np.full((2, WTOK), -1, dtype=np.int64)
    if core < 6:
        for w, q in enumerate((qa, qb)):
            wnode[w, :Q] = np.arange(q * Q, (q + 1) * Q)
    else:
        h = Q // 2
        for hw in range(2):  # window = half parity
            wnode[hw, :h] = np.arange(qa * Q + hw * h, qa * Q + (hw + 1) * h)
            wnode[hw, HTOK:HTOK + h] = np.arange(
                qb * Q + hw * h, qb * Q + (hw + 1) * h)

    # packed table [128, TCOL*D]: window w slot l -> partition w*64+l//392,
    # col l%392  (so DRAM flat row p*392+c == w*WTOK + l)
    xt = np.zeros((128, TCOL * D), dtype=x_bf.dtype)
    for w in range(2):
        valid = wnode[w] >= 0
        rowdat = np.zeros((WTOK, D), dtype=x_bf.dtype)
        rowdat[valid] = x_bf[wnode[w][valid]]
        rowdat[~valid, 0] = 1.0  # pad rows (never gathered)
        xt[w * 64:(w + 1) * 64, :] = rowdat.reshape(64, TCOL * D)

    swin, sloc = _win_local(core, src_l, sq)
    dwin, dloc = _win_local(core, dst_l, dq)

    ckey = swin * 2 + dwin            # 0=AA 1=AB 3=BB (2=BA: swap to AB)
    ba = ckey == 2
    if np.any(ba):  # orientation within diag windows: swap src/dst
        sloc[ba], dloc[ba] = dloc[ba].copy(), sloc[ba].copy()
        ckey[ba] = 1

    chunks_of = {1: [i for i, c in enumerate(COMBOS) if c == AB],
                 0: [i for i, c in enumerate(COMBOS) if c == AA],
                 3: [i for i, c in enumerate(COMBOS) if c == BB]}

    callidx = np.full((NCALL, CHUNK), -1, dtype=np.int16)
    counts = np.zeros(NCALL, dtype=np.int32)
    rows = np.empty(ne, dtype=np.int64)
    cols = np.empty(ne, dtype=np.int64)

    for key, chl in chunks_of.items():
        ids = np.nonzero(ckey == key)[0]
        cap = len(chl) * CHUNK
        if ids.size > cap:
            raise ValueError(f"combo {key} overflow: {ids.size} > {cap}")
        # (src band, dst row) sort for HBM row locality, then split evenly
        order = np.lexsort((dloc[ids], sloc[ids] // SBAND))
        ids = ids[order]
        nch = len(chl)
        base, rem = divmod(ids.size, nch)
        off = 0
        for k, ch in enumerate(chl):
            take = base + (1 if k < rem else 0)
            sel = ids[off:off + take]
            off += take
            v = sel.size
            nv = max(v, PADV)
            scall, dcall = 2 * ch, 2 * ch + 1
            callidx[scall, :v] = sloc[sel]
            callidx[dcall, :v] = dloc[sel]
            callidx[scall, v:nv] = 0
            callidx[dcall, v:nv] = 0
            counts[scall] = counts[dcall] = nv
            j = np.arange(v)
            rows[sel] = j % 128
            cols[sel] = ch * BLK + j // 128

    # wrap idx: element i of a call -> [i%16, call*ICOL + i//16], x8 replicate
    wrapped = callidx.reshape(NCALL, ICOL, 16).transpose(0, 2, 1)
    eidx16 = wrapped.transpose(1, 0, 2).reshape(16, NCALL * ICOL)
    eidx = np.tile(eidx16, (8, 1))

    return xt, np.ascontiguousarray(eidx), counts, rows, cols


def kernel(x, src, dst):
    global LAST_RESULTS
    import ml_dtypes
    from concourse.bass_utils import run_bass_kernel_spmd

    if "nc" not in _CACHE:
        _CACHE["nc"] = _build()
    nc = _CACHE["nc"]

    x_bf = np.asarray(x, dtype=np.float32).astype(ml_dtypes.bfloat16)
    src_i = np.asarray(src).astype(np.int64)
    dst_i = np.asarray(dst).astype(np.int64)

    qs = src_i // Q
    qd = dst_i // Q

    core_of = np.empty(E, dtype=np.int64)
    swap = np.zeros(E, dtype=bool)
    offdiag = qs != qd
    dmask = ~offdiag
    diag_lut = np.array([DIAG_CORE[q] for q in range(NQ)], dtype=np.int64)
    core_of[dmask] = diag_lut[qs[dmask]]
    for key, c in OFFDIAG_CORE.items():
        a, b = sorted(key)
        m = offdiag & (np.minimum(qs, qd) == a) & (np.maximum(qs, qd) == b)
        core_of[m] = c
        swap[m] = qs[m] != a        # orient src into window A's quarter
    ssrc = np.where(swap, dst_i, src_i)
    sdst = np.where(swap, src_i, dst_i)
    sqs = np.where(swap, qd, qs)
    sqd = np.where(swap, qs, qd)

    in_maps = []
    inv = []
    for i in range(CORES):
        m = np.nonzero(core_of == i)[0]
        xt, eidx, counts, rows, cols = _prepare_core(
            i, x_bf, ssrc[m], sdst[m], sqs[m], sqd[m])
        inv.append((m, rows, cols))
        in_maps.append({
            "xt": xt,
            "eidx": eidx,
            "cnt": np.ascontiguousarray(counts.reshape(1, NCALL)),
        })

    res = run_bass_kernel_spmd(nc, in_maps, core_ids=list(range(CORES)),
                               **RUN_KWARGS)
    LAST_RESULTS = res

    out = np.empty(E, dtype=np.float32)
    for i in range(CORES):
        tilev = np.asarray(res.results[i]["out"])
        m, rows, cols = inv[i]
        out[m] = tilev[rows, cols]
    return out.reshape(E, 1)


# revision 17
# speedup vs baseline: 1.3766x; 1.1075x over previous
"""Trainium2 Bass kernel for nn_CTRPredictor (gnn_message_passing).

score[e] = dot(normalize(x[src[e]]), normalize(x[dst[e]]))  for E edges.

Strategy (8 NeuronCores, SPMD):
  - Edges sharded: core i gets edges [i*80000, (i+1)*80000).
  - Each core L2-normalizes its 12500-node slice of x (ACT square, DVE
    reduce, sqrt, reciprocal, scale) to bf16; two half AllGathers replicate
    the normalized table to every core as 4 banks of 25000 rows (gathers on
    the first two banks overlap the second collective).
  - Host groups each core's edges by (src_bank, dst_bank) into 16 groups
    (so bank-local indices fit dma_gather's int16) with a fixed padded
    capacity per group (pad slots gather row 0 and are discarded).
  - Per group: dma_gather x_norm[src] and x_norm[dst] rows (256B bf16)
    across 4 SWDGE queues, DVE bf16 multiply + grouped reduce -> scores.
  - Host un-permutes scores back to edge order.
"""

import numpy as np

N = 100000
D = 128
E = 640000
CORES = 8
EPC = E // CORES          # 80000 edges per core
SLICE = N // CORES        # 12500 nodes normalized per core
QSL = SLICE // 4          # 3125-row quarter slices (AllGather chunks)
QCOL = 25                 # row-columns per quarter in the normalize layout
NBANK = 4
BANK = N // NBANK         # 25000 rows per stripe bank
NGRP = NBANK * NBANK      # 16 (src_bank, dst_bank) groups
GCAP = 5376               # padded edge capacity per group (42*128)
NCALLG = 2                # gather calls per group per endpoint
GCALL = GCAP // NCALLG    # 2688 indices per dma_gather call
CCOL = GCALL // 128       # 21 gathered row-columns per call
ICOL = GCALL // 16        # 168 index columns per call
NCALL = NGRP * NCALLG     # 32 slot-range calls (each does src + dst)
SCOL = NGRP * GCAP // 128  # 672 score columns
SP_NORM = 125             # partitions used in the normalize phase
RN = SLICE // SP_NORM     # 100 rows per partition in normalize phase

_CACHE = {}
LAST_RESULTS = None
RUN_KWARGS = {}  # extra kwargs for run_bass_kernel_spmd (used by test harness)


def _build():
    from concourse import bass, bacc, tile, mybir

    f32 = mybir.dt.float32
    bf16 = mybir.dt.bfloat16
    i16 = mybir.dt.int16
    i32 = mybir.dt.int32

    nc = bacc.Bacc("TRN2", target_bir_lowering=False, debug=False,
                   num_devices=CORES, num_swdge_queues=4,
                   dynamic_dma_scratch_size=40960)

    xsl_d = nc.dram_tensor("xsl", [SP_NORM, RN * D], f32, kind="ExternalInput")
    sidx_d = nc.dram_tensor("src_idx", [128, NCALL * ICOL], i16,
                            kind="ExternalInput")
    didx_d = nc.dram_tensor("dst_idx", [128, NCALL * ICOL], i16,
                            kind="ExternalInput")
    out_d = nc.dram_tensor("out", [128, SCOL], f32, kind="ExternalOutput")

    with tile.TileContext(nc) as tc:
        with tc.tile_pool(name="dram", bufs=1, space="DRAM") as dp, \
             tc.tile_pool(name="persist", bufs=1) as pp:

            # ---- index tables + score accumulator ----
            sidx = pp.tile([128, NCALL * ICOL], i16)
            didx = pp.tile([128, NCALL * ICOL], i16)
            nc.sync.dma_start(out=sidx[:, :], in_=sidx_d.ap())
            nc.sync.dma_start(out=didx[:, :], in_=didx_d.ap())
            score = pp.tile([128, SCOL], f32)

            # ---- phase 0: normalize this core's slice to bf16 ----
            banks = []
            with tc.tile_pool(name="ph0", bufs=1) as p0, \
                 tc.tile_pool(name="sqp", bufs=2) as sqp:
                xsl = p0.tile([SP_NORM, RN * D], f32)
                nc.sync.dma_start(out=xsl[:, :], in_=xsl_d.ap())
                ns = p0.tile([SP_NORM, RN], f32)
                rchunk = RN // 5
                for rc in range(5):
                    sq = sqp.tile([SP_NORM, rchunk * D], f32, tag="sq")
                    nc.scalar.activation(
                        out=sq[:, :],
                        in_=xsl[:, rc * rchunk * D:(rc + 1) * rchunk * D],
                        func=mybir.ActivationFunctionType.Square)
                    nc.vector.tensor_reduce(
                        out=ns[:, rc * rchunk:(rc + 1) * rchunk],
                        in_=sq[:, :].rearrange("p (r d) -> p r d", d=D),
                        axis=mybir.AxisListType.X,
                        op=mybir.AluOpType.add,
                    )
                nrm = p0.tile([SP_NORM, RN], f32)
                nc.scalar.activation(out=nrm[:, :], in_=ns[:, :],
                                     func=mybir.ActivationFunctionType.Sqrt)
                rns = p0.tile([SP_NORM, RN], f32)
                nc.vector.reciprocal(out=rns[:, :], in_=nrm[:, :])
                ntile = p0.tile([SP_NORM, RN * D], bf16)
                nc.vector.tensor_mul(
                    out=ntile[:, :].rearrange("p (r d) -> p r d", d=D),
                    in0=xsl[:, :].rearrange("p (r d) -> p r d", d=D),
                    in1=rns[:, :].unsqueeze(-1).to_broadcast(
                        [SP_NORM, RN, D]),
                )

                # ---- four quarter AllGathers of the normalized slice ----
                # Quarter q (columns [q*25,(q+1)*25) of ntile) gathers that
                # quarter of every core's slice; its output is bank q, so
                # gathers on early banks overlap the later collectives.
                for q in range(4):
                    agin = dp.tile([QSL, D], bf16, name=f"agin{q}")
                    htab = dp.tile([BANK, D], bf16, name=f"htab{q}",
                                   addr_space="Shared")
                    nc.sync.dma_start(
                        out=agin[:, :].rearrange("(p r) d -> p (r d)",
                                                 p=SP_NORM),
                        in_=ntile[:, q * QCOL * D:(q + 1) * QCOL * D],
                    )
                    nc.gpsimd.collective_compute(
                        "AllGather",
                        mybir.AluOpType.bypass,
                        replica_groups=[list(range(CORES))],
                        ins=[agin.opt()],
                        outs=[htab.opt()],
                    )
                    banks.append(htab[:, :])

            # ---- main loop: gathers on 4 queues, DVE dot per call ----
            # process groups in bank-availability order: a group needs banks
            # (a, b), and AllGather c completes before c+1 — order by max
            group_order = sorted(range(NGRP),
                                 key=lambda g: (max(g // NBANK, g % NBANK),
                                                g // NBANK, g % NBANK))
            with tc.tile_pool(name="ga", bufs=5) as ga, \
                 tc.tile_pool(name="gb", bufs=5) as gb:
                qn = 0
                for g in group_order:
                    ba, bb = g // NBANK, g % NBANK
                    for c in range(NCALLG):
                        call = g * NCALLG + c
                        col0 = call * ICOL
                        xs_t = ga.tile([128, CCOL * D], bf16, tag="A")
                        xd_t = gb.tile([128, CCOL * D], bf16, tag="B")
                        nc.gpsimd.dma_gather(
                            out_ap=xs_t[:, :].rearrange(
                                "p (c d) -> p c d", d=D),
                            in_ap=banks[ba][:, :],
                            idxs_ap=sidx[:, col0:col0 + ICOL],
                            num_idxs=GCALL, num_idxs_reg=GCALL, elem_size=D,
                            single_packet=False, queue_num=qn % 4,
                        )
                        qn += 1
                        nc.gpsimd.dma_gather(
                            out_ap=xd_t[:, :].rearrange(
                                "p (c d) -> p c d", d=D),
                            in_ap=banks[bb][:, :],
                            idxs_ap=didx[:, col0:col0 + ICOL],
                            num_idxs=GCALL, num_idxs_reg=GCALL, elem_size=D,
                            single_packet=False, queue_num=qn % 4,
                        )
                        qn += 1
                        nc.vector.tensor_mul(out=xs_t[:, :], in0=xs_t[:, :],
                                             in1=xd_t[:, :])
                        sc0 = call * CCOL
                        nc.vector.tensor_reduce(
                            out=score[:, sc0:sc0 + CCOL],
                            in_=xs_t[:, :].rearrange("p (c d) -> p c d", d=D),
                            axis=mybir.AxisListType.X,
                            op=mybir.AluOpType.add,
                        )

                nc.sync.dma_start(out=out_d.ap(), in_=score[:, :])

    nc.compile()
    return nc


def _node_map(n):
    """node id -> (bank, bank-local index) for the quarter-AllGather layout.

    Slice-local node j sits at ntile[j % 125, (j // 3125)*25 + (j % 3125)//125]
    => agin_q row (p*25 + rr) = node q*3125 + rr*125 + p of the slice, and
    core r's quarter lands at htab_q rows [r*3125, (r+1)*3125).
    """
    r = n // SLICE
    rem = n - r * SLICE
    q = rem // QSL
    w = rem - q * QSL
    rr = w // SP_NORM
    p = w - rr * SP_NORM
    bank = q
    local = r * QSL + p * QCOL + rr
    return bank, local


def _wrap_idx(flat):
    """[GCALL] int16 -> [128, ICOL] in dma_gather's 16-partition wrap."""
    blk = flat.reshape(ICOL, 16).T  # index i at [i%16, i//16]
    return np.tile(blk, (8, 1))


def _prepare_core(src_l, dst_l):
    """Group one core's edges by bank pair; build index tilings + inverse."""
    sb, sl = _node_map(src_l)
    db, dl = _node_map(dst_l)
    key = sb * NBANK + db
    order = np.argsort(key, kind="stable")
    sizes = np.bincount(key, minlength=NGRP)
    if sizes.max() > GCAP:
        raise ValueError(f"group overflow: {sizes.max()} > {GCAP}")
    if sizes.min() <= GCALL:
        raise ValueError(f"group underflow: {sizes.min()} <= {GCALL}")

    sidx = np.zeros((128, NCALL * ICOL), dtype=np.int16)
    didx = np.zeros((128, NCALL * ICOL), dtype=np.int16)
    # inverse: score of edge order[...] lives at [row, col] of out tile
    rows = np.empty(EPC, dtype=np.int64)
    cols = np.empty(EPC, dtype=np.int64)
    off = 0
    for g in range(NGRP):
        ids = order[off:off + sizes[g]]
        off += sizes[g]
        # ascending src addresses give the src-side gather descriptors
        # HBM locality (the dst side stays random)
        ids = ids[np.argsort(sl[ids], kind="stable")]
        s_pad = np.zeros(GCAP, dtype=np.int16)
        d_pad = np.zeros(GCAP, dtype=np.int16)
        s_pad[:ids.size] = sl[ids]
        d_pad[:ids.size] = dl[ids]
        for c in range(NCALLG):
            call = g * NCALLG + c
            col0 = call * ICOL
            seg = slice(c * GCALL, (c + 1) * GCALL)
            sidx[:, col0:col0 + ICOL] = _wrap_idx(s_pad[seg])
            didx[:, col0:col0 + ICOL] = _wrap_idx(d_pad[seg])
        j = np.arange(ids.size)
        rows[ids] = j % 128
        cols[ids] = g * (GCAP // 128) + j // 128
    return sidx, didx, rows, cols


def kernel(x, src, dst):
    global LAST_RESULTS
    from concourse.bass_utils import run_bass_kernel_spmd

    if "nc" not in _CACHE:
        _CACHE["nc"] = _build()
    nc = _CACHE["nc"]

    x32 = np.ascontiguousarray(np.asarray(x, dtype=np.float32))
    src_i = np.asarray(src).astype(np.int64)
    dst_i = np.asarray(dst).astype(np.int64)

    in_maps = []
    inv = []
    for i in range(CORES):
        sidx, didx, rows, cols = _prepare_core(
            src_i[i * EPC:(i + 1) * EPC], dst_i[i * EPC:(i + 1) * EPC])
        inv.append((rows, cols))
        in_maps.append({
            "xsl": np.ascontiguousarray(
                x32[i * SLICE:(i + 1) * SLICE]
                .reshape(4, QCOL, SP_NORM, D).transpose(2, 0, 1, 3)
                .reshape(SP_NORM, RN * D)),
            "src_idx": np.ascontiguousarray(sidx),
            "dst_idx": np.ascontiguousarray(didx),
        })

    res = run_bass_kernel_spmd(nc, in_maps, core_ids=list(range(CORES)),
                               **RUN_KWARGS)
    LAST_RESULTS = res

    out = np.empty(E, dtype=np.float32)
    for i in range(CORES):
        tilev = np.asarray(res.results[i]["out"])
        rows, cols = inv[i]
        out[i * EPC:(i + 1) * EPC] = tilev[rows, cols]
    return out.reshape(E, 1)



# revision 18
# speedup vs baseline: 1.4624x; 1.0623x over previous
"""Trainium2 Bass kernel for nn_CTRPredictor (gnn_message_passing).

score[e] = dot(normalize(x[src[e]]), normalize(x[dst[e]]))  for E edges.

Strategy (8 NeuronCores, SPMD):
  - Edges sharded: core i gets edges [i*80000, (i+1)*80000).
  - Each core L2-normalizes its 12500-node slice of x (ACT square, DVE
    reduce, sqrt, reciprocal, scale) to bf16; two half AllGathers replicate
    the normalized table to every core as 4 banks of 25000 rows (gathers on
    the first two banks overlap the second collective).
  - Host groups each core's edges by (src_bank, dst_bank) into 16 groups
    (so bank-local indices fit dma_gather's int16) with a fixed padded
    capacity per group (pad slots gather row 0 and are discarded).
  - Per group: dma_gather x_norm[src] and x_norm[dst] rows (256B bf16)
    across 4 SWDGE queues, DVE bf16 multiply + grouped reduce -> scores.
  - Host un-permutes scores back to edge order.
"""

import numpy as np

N = 100000
D = 128
E = 640000
CORES = 8
EPC = E // CORES          # 80000 edges per core
SLICE = N // CORES        # 12500 nodes normalized per core
QSL = SLICE // 4          # 3125-row quarter slices (AllGather chunks)
QCOL = 25                 # row-columns per quarter in the normalize layout
NBANK = 4
BANK = N // NBANK         # 25000 rows per stripe bank
NGRP = NBANK * NBANK      # 16 (src_bank, dst_bank) groups
GCAP = 5376               # padded edge capacity per group (42*128)
NCALLG = 2                # gather calls per group per endpoint
GCALL = GCAP // NCALLG    # 2688 indices per dma_gather call
CCOL = GCALL // 128       # 21 gathered row-columns per call
ICOL = GCALL // 16        # 168 index columns per call
NCALL = NGRP * NCALLG     # 32 slot-range calls (each does src + dst)
SCOL = NGRP * GCAP // 128  # 672 score columns
SP_NORM = 125             # partitions used in the normalize phase
RN = SLICE // SP_NORM     # 100 rows per partition in normalize phase

_CACHE = {}
LAST_RESULTS = None
RUN_KWARGS = {}  # extra kwargs for run_bass_kernel_spmd (used by test harness)


def _build():
    from concourse import bass, bacc, tile, mybir

    f32 = mybir.dt.float32
    bf16 = mybir.dt.bfloat16
    i16 = mybir.dt.int16
    i32 = mybir.dt.int32

    nc = bacc.Bacc("TRN2", target_bir_lowering=False, debug=False,
                   num_devices=CORES, num_swdge_queues=4,
                   dynamic_dma_scratch_size=40960)

    xsl_d = nc.dram_tensor("xsl", [SP_NORM, RN * D], f32, kind="ExternalInput")
    sidx_d = nc.dram_tensor("src_idx", [128, NCALL * ICOL], i16,
                            kind="ExternalInput")
    didx_d = nc.dram_tensor("dst_idx", [128, NCALL * ICOL], i16,
                            kind="ExternalInput")
    out_d = nc.dram_tensor("out", [128, SCOL], f32, kind="ExternalOutput")

    with tile.TileContext(nc) as tc:
        with tc.tile_pool(name="dram", bufs=1, space="DRAM") as dp, \
             tc.tile_pool(name="persist", bufs=1) as pp:

            # ---- index tables + score accumulator ----
            sidx = pp.tile([128, NCALL * ICOL], i16)
            didx = pp.tile([128, NCALL * ICOL], i16)
            nc.sync.dma_start(out=sidx[:, :], in_=sidx_d.ap())
            nc.sync.dma_start(out=didx[:, :], in_=didx_d.ap())
            score = pp.tile([128, SCOL], f32)

            # ---- phase 0: normalize this core's slice to bf16 ----
            # fully per-quarter pipeline: each quarter is loaded,
            # normalized, and AllGathered independently so bank q is
            # available without waiting for quarters > q.
            banks = []
            with tc.tile_pool(name="ph0", bufs=1) as p0, \
                 tc.tile_pool(name="sqp", bufs=2) as sqp:
                xsl = p0.tile([SP_NORM, RN * D], f32)
                ns = p0.tile([SP_NORM, RN], f32)
                rns = p0.tile([SP_NORM, RN], f32)
                ntile = p0.tile([SP_NORM, RN * D], bf16)
                for q in range(4):
                    c0 = q * QCOL
                    xseg = xsl[:, c0 * D:(c0 + QCOL) * D]
                    nc.sync.dma_start(
                        out=xseg,
                        in_=xsl_d.ap()[:, c0 * D:(c0 + QCOL) * D])
                    sq = sqp.tile([SP_NORM, QCOL * D], f32, tag="sq")
                    nc.scalar.activation(
                        out=sq[:, :], in_=xseg,
                        func=mybir.ActivationFunctionType.Square)
                    nc.vector.tensor_reduce(
                        out=ns[:, c0:c0 + QCOL],
                        in_=sq[:, :].rearrange("p (r d) -> p r d", d=D),
                        axis=mybir.AxisListType.X,
                        op=mybir.AluOpType.add,
                    )
                    nc.scalar.activation(
                        out=ns[:, c0:c0 + QCOL], in_=ns[:, c0:c0 + QCOL],
                        func=mybir.ActivationFunctionType.Sqrt)
                    nc.vector.reciprocal(out=rns[:, c0:c0 + QCOL],
                                         in_=ns[:, c0:c0 + QCOL])
                    nc.vector.tensor_mul(
                        out=ntile[:, c0 * D:(c0 + QCOL) * D].rearrange(
                            "p (r d) -> p r d", d=D),
                        in0=xseg.rearrange("p (r d) -> p r d", d=D),
                        in1=rns[:, c0:c0 + QCOL].unsqueeze(-1).to_broadcast(
                            [SP_NORM, QCOL, D]),
                    )
                    agin = dp.tile([QSL, D], bf16, name=f"agin{q}")
                    htab = dp.tile([BANK, D], bf16, name=f"htab{q}",
                                   addr_space="Shared")
                    nc.sync.dma_start(
                        out=agin[:, :].rearrange("(p r) d -> p (r d)",
                                                 p=SP_NORM),
                        in_=ntile[:, c0 * D:(c0 + QCOL) * D],
                    )
                    nc.gpsimd.collective_compute(
                        "AllGather",
                        mybir.AluOpType.bypass,
                        replica_groups=[list(range(CORES))],
                        ins=[agin.opt()],
                        outs=[htab.opt()],
                    )
                    banks.append(htab[:, :])

            # ---- main loop: gathers on 4 queues, DVE dot per call ----
            # process groups in bank-availability order: a group needs banks
            # (a, b), and AllGather c completes before c+1 — order by max
            group_order = sorted(range(NGRP),
                                 key=lambda g: (max(g // NBANK, g % NBANK),
                                                g // NBANK, g % NBANK))
            with tc.tile_pool(name="ga", bufs=5) as ga, \
                 tc.tile_pool(name="gb", bufs=5) as gb:
                qn = 0
                for g in group_order:
                    ba, bb = g // NBANK, g % NBANK
                    for c in range(NCALLG):
                        call = g * NCALLG + c
                        col0 = call * ICOL
                        xs_t = ga.tile([128, CCOL * D], bf16, tag="A")
                        xd_t = gb.tile([128, CCOL * D], bf16, tag="B")
                        nc.gpsimd.dma_gather(
                            out_ap=xs_t[:, :].rearrange(
                                "p (c d) -> p c d", d=D),
                            in_ap=banks[ba][:, :],
                            idxs_ap=sidx[:, col0:col0 + ICOL],
                            num_idxs=GCALL, num_idxs_reg=GCALL, elem_size=D,
                            single_packet=False, queue_num=qn % 4,
                        )
                        qn += 1
                        nc.gpsimd.dma_gather(
                            out_ap=xd_t[:, :].rearrange(
                                "p (c d) -> p c d", d=D),
                            in_ap=banks[bb][:, :],
                            idxs_ap=didx[:, col0:col0 + ICOL],
                            num_idxs=GCALL, num_idxs_reg=GCALL, elem_size=D,
                            single_packet=False, queue_num=qn % 4,
                        )
                        qn += 1
                        nc.vector.tensor_mul(out=xs_t[:, :], in0=xs_t[:, :],
                                             in1=xd_t[:, :])
                        sc0 = call * CCOL
                        nc.vector.tensor_reduce(
                            out=score[:, sc0:sc0 + CCOL],
                            in_=xs_t[:, :].rearrange("p (c d) -> p c d", d=D),
                            axis=mybir.AxisListType.X,
                            op=mybir.AluOpType.add,
                        )

                nc.sync.dma_start(out=out_d.ap(), in_=score[:, :])

    nc.compile()
    return nc


def _node_map(n):
    """node id -> (bank, bank-local index) for the quarter-AllGather layout.

    Slice-local node j sits at ntile[j % 125, (j // 3125)*25 + (j % 3125)//125]
    => agin_q row (p*25 + rr) = node q*3125 + rr*125 + p of the slice, and
    core r's quarter lands at htab_q rows [r*3125, (r+1)*3125).
    """
    r = n // SLICE
    rem = n - r * SLICE
    q = rem // QSL
    w = rem - q * QSL
    rr = w // SP_NORM
    p = w - rr * SP_NORM
    bank = q
    local = r * QSL + p * QCOL + rr
    return bank, local


def _wrap_idx(flat):
    """[GCALL] int16 -> [128, ICOL] in dma_gather's 16-partition wrap."""
    blk = flat.reshape(ICOL, 16).T  # index i at [i%16, i//16]
    return np.tile(blk, (8, 1))


def _prepare_core(src_l, dst_l):
    """Group one core's edges by bank pair; build index tilings + inverse."""
    sb, sl = _node_map(src_l)
    db, dl = _node_map(dst_l)
    key = sb * NBANK + db
    order = np.argsort(key, kind="stable")
    sizes = np.bincount(key, minlength=NGRP)
    if sizes.max() > GCAP:
        raise ValueError(f"group overflow: {sizes.max()} > {GCAP}")
    if sizes.min() <= GCALL:
        raise ValueError(f"group underflow: {sizes.min()} <= {GCALL}")

    sidx = np.zeros((128, NCALL * ICOL), dtype=np.int16)
    didx = np.zeros((128, NCALL * ICOL), dtype=np.int16)
    # inverse: score of edge order[...] lives at [row, col] of out tile
    rows = np.empty(EPC, dtype=np.int64)
    cols = np.empty(EPC, dtype=np.int64)
    off = 0
    for g in range(NGRP):
        ids = order[off:off + sizes[g]]
        off += sizes[g]
        # ascending src addresses give the src-side gather descriptors
        # HBM locality (the dst side stays random)
        ids = ids[np.argsort(sl[ids], kind="stable")]
        s_pad = np.zeros(GCAP, dtype=np.int16)
        d_pad = np.zeros(GCAP, dtype=np.int16)
        s_pad[:ids.size] = sl[ids]
        d_pad[:ids.size] = dl[ids]
        for c in range(NCALLG):
            call = g * NCALLG + c
            col0 = call * ICOL
            seg = slice(c * GCALL, (c + 1) * GCALL)
            sidx[:, col0:col0 + ICOL] = _wrap_idx(s_pad[seg])
            didx[:, col0:col0 + ICOL] = _wrap_idx(d_pad[seg])
        j = np.arange(ids.size)
        rows[ids] = j % 128
        cols[ids] = g * (GCAP // 128) + j // 128
    return sidx, didx, rows, cols


def kernel(x, src, dst):
    global LAST_RESULTS
    from concourse.bass_utils import run_bass_kernel_spmd

    if "nc" not in _CACHE:
        _CACHE["nc"] = _build()
    nc = _CACHE["nc"]

    x32 = np.ascontiguousarray(np.asarray(x, dtype=np.float32))
    src_i = np.asarray(src).astype(np.int64)
    dst_i = np.asarray(dst).astype(np.int64)

    in_maps = []
    inv = []
    for i in range(CORES):
        sidx, didx, rows, cols = _prepare_core(
            src_i[i * EPC:(i + 1) * EPC], dst_i[i * EPC:(i + 1) * EPC])
        inv.append((rows, cols))
        in_maps.append({
            "xsl": np.ascontiguousarray(
                x32[i * SLICE:(i + 1) * SLICE]
                .reshape(4, QCOL, SP_NORM, D).transpose(2, 0, 1, 3)
                .reshape(SP_NORM, RN * D)),
            "src_idx": np.ascontiguousarray(sidx),
            "dst_idx": np.ascontiguousarray(didx),
        })

    res = run_bass_kernel_spmd(nc, in_maps, core_ids=list(range(CORES)),
                               **RUN_KWARGS)
    LAST_RESULTS = res

    out = np.empty(E, dtype=np.float32)
    for i in range(CORES):
        tilev = np.asarray(res.results[i]["out"])
        rows, cols = inv[i]
        out[i * EPC:(i + 1) * EPC] = tilev[rows, cols]
    return out.reshape(E, 1)



# revision 20
# speedup vs baseline: 1.8443x; 1.2611x over previous
"""Trainium2 Bass kernel for nn_CTRPredictor (gnn_message_passing).

score[e] = dot(normalize(x[src[e]]), normalize(x[dst[e]]))  for E edges.

Strategy (8 NeuronCores, SPMD):
  - Edges sharded: core i gets edges [i*80000, (i+1)*80000).
  - Each core L2-normalizes its 12500-node slice of x (ACT square, DVE
    reduce, sqrt, reciprocal, scale) to bf16; two half AllGathers replicate
    the normalized table to every core as 4 banks of 25000 rows (gathers on
    the first two banks overlap the second collective).
  - Host groups each core's edges by (src_bank, dst_bank) into 16 groups
    (so bank-local indices fit dma_gather's int16) with a fixed padded
    capacity per group (pad slots gather row 0 and are discarded).
  - Per group: dma_gather x_norm[src] and x_norm[dst] rows (256B bf16)
    across 4 SWDGE queues, DVE bf16 multiply + grouped reduce -> scores.
  - Host un-permutes scores back to edge order.
"""

import numpy as np

N = 100000
D = 128
E = 640000
CORES = 8
EPC = E // CORES          # 80000 edges per core
SLICE = N // CORES        # 12500 nodes normalized per core
QSL = SLICE // 4          # 3125-row quarter slices (AllGather chunks)
QCOL = 25                 # row-columns per quarter in the normalize layout
NBANK = 4
BANK = N // NBANK         # 25000 rows per stripe bank
NGRP = NBANK * NBANK      # 16 (src_bank, dst_bank) groups
GCAP = 5376               # padded edge capacity per group (42*128)
NCALLG = 2                # gather calls per group per endpoint
GCALL = GCAP // NCALLG    # 2688 indices per dma_gather call
CCOL = GCALL // 128       # 21 gathered row-columns per call
ICOL = GCALL // 16        # 168 index columns per call
NCALL = NGRP * NCALLG     # 32 slot-range calls (each does src + dst)
SCOL = NGRP * GCAP // 128  # 672 score columns
SP_NORM = 125             # partitions used in the normalize phase
RN = SLICE // SP_NORM     # 100 rows per partition in normalize phase

_CACHE = {}
LAST_RESULTS = None
RUN_KWARGS = {}  # extra kwargs for run_bass_kernel_spmd (used by test harness)


def _build():
    from concourse import bass, bacc, tile, mybir

    f32 = mybir.dt.float32
    bf16 = mybir.dt.bfloat16
    i16 = mybir.dt.int16
    i32 = mybir.dt.int32

    nc = bacc.Bacc("TRN2", target_bir_lowering=False, debug=False,
                   num_devices=CORES, num_swdge_queues=4,
                   dynamic_dma_scratch_size=40960)

    xsl_d = nc.dram_tensor("xsl", [SP_NORM, RN * D], f32, kind="ExternalInput")
    sidx_d = nc.dram_tensor("src_idx", [128, NCALL * ICOL], i16,
                            kind="ExternalInput")
    didx_d = nc.dram_tensor("dst_idx", [128, NCALL * ICOL], i16,
                            kind="ExternalInput")
    cnt_d = nc.dram_tensor("cnt", [1, NCALL], i32, kind="ExternalInput")
    out_d = nc.dram_tensor("out", [128, SCOL], f32, kind="ExternalOutput")

    with tile.TileContext(nc) as tc:
        with tc.tile_pool(name="dram", bufs=1, space="DRAM") as dp, \
             tc.tile_pool(name="persist", bufs=1) as pp:

            # ---- index tables + score accumulator ----
            sidx = pp.tile([128, NCALL * ICOL], i16)
            didx = pp.tile([128, NCALL * ICOL], i16)
            cnt = pp.tile([1, NCALL], i32)
            nc.sync.dma_start(out=sidx[:, :], in_=sidx_d.ap())
            nc.sync.dma_start(out=didx[:, :], in_=didx_d.ap())
            nc.sync.dma_start(out=cnt[:, :], in_=cnt_d.ap())
            score = pp.tile([128, SCOL], f32)

            # ---- phase 0: normalize this core's slice to bf16 ----
            # fully per-quarter pipeline: each quarter is loaded,
            # normalized, and AllGathered independently so bank q is
            # available without waiting for quarters > q.
            banks = []
            with tc.tile_pool(name="ph0", bufs=1) as p0, \
                 tc.tile_pool(name="sqp", bufs=2) as sqp:
                xsl = p0.tile([SP_NORM, RN * D], f32)
                ns = p0.tile([SP_NORM, RN], f32)
                rns = p0.tile([SP_NORM, RN], f32)
                ntile = p0.tile([SP_NORM, RN * D], bf16)
                for q in range(4):
                    c0 = q * QCOL
                    xseg = xsl[:, c0 * D:(c0 + QCOL) * D]
                    nc.sync.dma_start(
                        out=xseg,
                        in_=xsl_d.ap()[:, c0 * D:(c0 + QCOL) * D])
                    sq = sqp.tile([SP_NORM, QCOL * D], f32, tag="sq")
                    nc.scalar.activation(
                        out=sq[:, :], in_=xseg,
                        func=mybir.ActivationFunctionType.Square)
                    nc.vector.tensor_reduce(
                        out=ns[:, c0:c0 + QCOL],
                        in_=sq[:, :].rearrange("p (r d) -> p r d", d=D),
                        axis=mybir.AxisListType.X,
                        op=mybir.AluOpType.add,
                    )
                    nc.scalar.activation(
                        out=ns[:, c0:c0 + QCOL], in_=ns[:, c0:c0 + QCOL],
                        func=mybir.ActivationFunctionType.Sqrt)
                    nc.vector.reciprocal(out=rns[:, c0:c0 + QCOL],
                                         in_=ns[:, c0:c0 + QCOL])
                    nc.vector.tensor_mul(
                        out=ntile[:, c0 * D:(c0 + QCOL) * D].rearrange(
                            "p (r d) -> p r d", d=D),
                        in0=xseg.rearrange("p (r d) -> p r d", d=D),
                        in1=rns[:, c0:c0 + QCOL].unsqueeze(-1).to_broadcast(
                            [SP_NORM, QCOL, D]),
                    )
                    agin = dp.tile([QSL, D], bf16, name=f"agin{q}")
                    htab = dp.tile([BANK, D], bf16, name=f"htab{q}",
                                   addr_space="Shared")
                    nc.sync.dma_start(
                        out=agin[:, :].rearrange("(p r) d -> p (r d)",
                                                 p=SP_NORM),
                        in_=ntile[:, c0 * D:(c0 + QCOL) * D],
                    )
                    nc.gpsimd.collective_compute(
                        "AllGather",
                        mybir.AluOpType.bypass,
                        replica_groups=[list(range(CORES))],
                        ins=[agin.opt()],
                        outs=[htab.opt()],
                    )
                    banks.append(htab[:, :])

            # ---- main loop: gathers on 4 queues, DVE dot per call ----
            # process groups in bank-availability order: a group needs banks
            # (a, b), and AllGather c completes before c+1 — order by max
            group_order = sorted(range(NGRP),
                                 key=lambda g: (max(g // NBANK, g % NBANK),
                                                g // NBANK, g % NBANK))
            creg = nc.gpsimd.alloc_register("cnt_reg")
            with tc.tile_pool(name="ga", bufs=5) as ga, \
                 tc.tile_pool(name="gb", bufs=5) as gb:
                qn = 0
                for g in group_order:
                    ba, bb = g // NBANK, g % NBANK
                    for c in range(NCALLG):
                        call = g * NCALLG + c
                        col0 = call * ICOL
                        xs_t = ga.tile([128, CCOL * D], bf16, tag="A")
                        xd_t = gb.tile([128, CCOL * D], bf16, tag="B")
                        nc.gpsimd.reg_load(creg, cnt[0:1, call:call + 1])
                        nc.gpsimd.dma_gather(
                            out_ap=xs_t[:, :].rearrange(
                                "p (c d) -> p c d", d=D),
                            in_ap=banks[ba][:, :],
                            idxs_ap=sidx[:, col0:col0 + ICOL],
                            num_idxs=GCALL, num_idxs_reg=creg, elem_size=D,
                            single_packet=False, queue_num=qn % 4,
                        )
                        qn += 1
                        nc.gpsimd.dma_gather(
                            out_ap=xd_t[:, :].rearrange(
                                "p (c d) -> p c d", d=D),
                            in_ap=banks[bb][:, :],
                            idxs_ap=didx[:, col0:col0 + ICOL],
                            num_idxs=GCALL, num_idxs_reg=creg, elem_size=D,
                            single_packet=False, queue_num=qn % 4,
                        )
                        qn += 1
                        nc.vector.tensor_mul(out=xs_t[:, :], in0=xs_t[:, :],
                                             in1=xd_t[:, :])
                        sc0 = call * CCOL
                        nc.vector.tensor_reduce(
                            out=score[:, sc0:sc0 + CCOL],
                            in_=xs_t[:, :].rearrange("p (c d) -> p c d", d=D),
                            axis=mybir.AxisListType.X,
                            op=mybir.AluOpType.add,
                        )

                nc.sync.dma_start(out=out_d.ap(), in_=score[:, :])

    nc.compile()
    return nc


def _node_map(n):
    """node id -> (bank, bank-local index) for the quarter-AllGather layout.

    Slice-local node j sits at ntile[j % 125, (j // 3125)*25 + (j % 3125)//125]
    => agin_q row (p*25 + rr) = node q*3125 + rr*125 + p of the slice, and
    core r's quarter lands at htab_q rows [r*3125, (r+1)*3125).
    """
    r = n // SLICE
    rem = n - r * SLICE
    q = rem // QSL
    w = rem - q * QSL
    rr = w // SP_NORM
    p = w - rr * SP_NORM
    bank = q
    local = r * QSL + p * QCOL + rr
    return bank, local


def _wrap_idx(flat):
    """[GCALL] int16 -> [128, ICOL] in dma_gather's 16-partition wrap."""
    blk = flat.reshape(ICOL, 16).T  # index i at [i%16, i//16]
    return np.tile(blk, (8, 1))


def _prepare_core(src_l, dst_l):
    """Group one core's edges by bank pair; build index tilings + inverse."""
    sb, sl = _node_map(src_l)
    db, dl = _node_map(dst_l)
    key = sb * NBANK + db
    order = np.argsort(key, kind="stable")
    sizes = np.bincount(key, minlength=NGRP)
    if sizes.max() > GCAP:
        raise ValueError(f"group overflow: {sizes.max()} > {GCAP}")
    if sizes.min() <= GCALL + 128:
        raise ValueError(f"group underflow: {sizes.min()} <= {GCALL + 128}")

    sidx = np.zeros((128, NCALL * ICOL), dtype=np.int16)
    didx = np.zeros((128, NCALL * ICOL), dtype=np.int16)
    counts = np.zeros(NCALL, dtype=np.int32)
    # inverse: score of edge order[...] lives at [row, col] of out tile
    rows = np.empty(EPC, dtype=np.int64)
    cols = np.empty(EPC, dtype=np.int64)
    off = 0
    for g in range(NGRP):
        ids = order[off:off + sizes[g]]
        off += sizes[g]
        # ascending src addresses give the src-side gather descriptors
        # HBM locality (the dst side stays random)
        ids = ids[np.argsort(sl[ids], kind="stable")]
        s_pad = np.full(GCAP, -1, dtype=np.int16)
        d_pad = np.full(GCAP, -1, dtype=np.int16)
        s_pad[:ids.size] = sl[ids]
        d_pad[:ids.size] = dl[ids]
        for c in range(NCALLG):
            call = g * NCALLG + c
            col0 = call * ICOL
            seg = slice(c * GCALL, (c + 1) * GCALL)
            sidx[:, col0:col0 + ICOL] = _wrap_idx(s_pad[seg])
            didx[:, col0:col0 + ICOL] = _wrap_idx(d_pad[seg])
            counts[call] = min(max(int(ids.size) - c * GCALL, 0), GCALL)
        j = np.arange(ids.size)
        rows[ids] = j % 128
        cols[ids] = g * (GCAP // 128) + j // 128
    return sidx, didx, counts, rows, cols


def kernel(x, src, dst):
    global LAST_RESULTS
    from concourse.bass_utils import run_bass_kernel_spmd

    if "nc" not in _CACHE:
        _CACHE["nc"] = _build()
    nc = _CACHE["nc"]

    x32 = np.ascontiguousarray(np.asarray(x, dtype=np.float32))
    src_i = np.asarray(src).astype(np.int64)
    dst_i = np.asarray(dst).astype(np.int64)

    in_maps = []
    inv = []
    for i in range(CORES):
        sidx, didx, counts, rows, cols = _prepare_core(
            src_i[i * EPC:(i + 1) * EPC], dst_i[i * EPC:(i + 1) * EPC])
        inv.append((rows, cols))
        in_maps.append({
            "xsl": np.ascontiguousarray(
                x32[i * SLICE:(i + 1) * SLICE]
                .reshape(4, QCOL, SP_NORM, D).transpose(2, 0, 1, 3)
                .reshape(SP_NORM, RN * D)),
            "src_idx": np.ascontiguousarray(sidx),
            "dst_idx": np.ascontiguousarray(didx),
            "cnt": np.ascontiguousarray(counts.reshape(1, NCALL)),
        })

    res = run_bass_kernel_spmd(nc, in_maps, core_ids=list(range(CORES)),
                               **RUN_KWARGS)
    LAST_RESULTS = res

    out = np.empty(E, dtype=np.float32)
    for i in range(CORES):
        tilev = np.asarray(res.results[i]["out"])
        rows, cols = inv[i]
        out[i * EPC:(i + 1) * EPC] = tilev[rows, cols]
    return out.reshape(E, 1)

